# revision 1
# baseline (speedup 1.0000x reference)
"""Trainium2 Bass kernel for nn_Detector (region-sum pooling + softmax).

The reference computes softmax(x.reshape(B, H*W) @ filt) where filt is a
fixed 0/1 mask selecting 10 disjoint 113x113 rectangular regions of the
1024x1024 image.  The dense GEMM is really a sparse pooling: out[b, k]
is the sum of x[b] over region k.  Only ~12% of x is ever needed, so we
DMA exactly the 10 regions per image instead of streaming all 512 MB.

Distribution: data-parallel over batch, 8 NeuronCores x 16 images each.

v3: SDMA descriptors under 512 B pay a 2x read-modify-write penalty
(cost model: latency_multiplier=2 below 512 B; measured 40 ns/desc for
the natural 452 B region-row chunks = exactly 452*2/22.5).  So each
region row is loaded through a 128-column (512 B) window aligned down
to a 64 B DRAM boundary: descriptor rate doubles to 22.8 ns, and the
stream lands on the per-core HBM roofline (~9.25 MB at 358 GB/s).
Multi-queue splitting does NOT help (v2: HWDGE emits 1-desc packets at
~55 GB/s/queue and starves SWDGE; the 16 SDMA engines process packets
serially), so all bulk loads stay on the single SWDGE queue; only the
tiny remainder-row DMAs ride the HWDGE queues in parallel.

Per core and region, one SWDGE (gpsimd) DMA loads rows r0..r0+111 for
all 16 images: DRAM side x[:, r0:r0+112, w0:w0+128]; SBUF side is
[128, 14, 128] with partition = (batch, row-octet).  Both enumerate
elements in the same order (SWDGE needs monotonic APs).  The 113th row
of each region is batched into 5 stepped-slice HWDGE DMAs (regions
sharing a column block lie at uniform row stride).

Compute: VectorE reduces the 113 live columns of each region tile to a
[128, 1] partial; a per-region TensorE matmul with a 0/1 block
indicator [128, 16] contracts the 8 octets per batch into PSUM column
k; VectorE adds the remainder-row partials; ScalarE does the
numerically-stable softmax.
"""

import numpy as np

import concourse.bass as bass
import concourse.tile as tile
from concourse import bacc, mybir
from concourse.bass_utils import run_bass_kernel_spmd

# Problem geometry — fixed by the reference's _build_filter(1024, 1024).
B, H, W = 128, 1024, 1024
S = 113  # min(1024 // 9, 1024 // 7)
REGIONS = [(2, 1), (2, 4), (2, 7), (4, 1), (4, 3), (4, 5), (4, 7), (6, 1), (6, 4), (6, 7)]
K = len(REGIONS)
N_CORES = 8
BPC = B // N_CORES  # images per core
F32 = mybir.dt.float32
OCT, GR = 8, 14  # 112 of the 113 region rows = 8 octets x 14 rows
PADW = 128  # 512 B descriptors (>= 512 avoids the SDMA RMW 2x penalty)


def win(cb):
    """64 B-aligned 128-col window covering col block cb; returns (w0, off)."""
    c0 = cb * S
    w0 = (c0 // 16) * 16
    return w0, c0 - w0


# Remainder-row groups: regions sharing a column block lie at uniform row
# stride, so one stepped-slice DMA covers each group.
#   (row_slice, col_block, [region ks], queue) with row = rb*S + 112.
REM_GROUPS = [
    ((338, 791, 226), 1, [0, 3, 7], "sync"),    # c=1: rows 338,564,790
    ((338, 791, 226), 7, [2, 6, 9], "scalar"),  # c=7
    ((338, 791, 452), 4, [1, 8], "sync"),       # c=4: rows 338,790
    ((564, 565, 1), 3, [4], "scalar"),          # c=3: row 564
    ((564, 565, 1), 5, [5], "sync"),            # c=5
]


def build_nc():
    nc = bacc.Bacc("TRN2", target_bir_lowering=False, debug=False)
    x = nc.declare_dram_parameter("x", [BPC, H, W], F32, isOutput=False)
    blk_d = nc.declare_dram_parameter("blk", [128, BPC], F32, isOutput=False)
    out = nc.declare_dram_parameter("out", [BPC, K], F32, isOutput=True)

    with tile.TileContext(nc) as tc:
        with (
            tc.tile_pool(name="reg", bufs=1) as rpool,
            tc.tile_pool(name="mp", bufs=1) as mpool,
            tc.tile_pool(name="small", bufs=1) as spool,
            tc.tile_pool(name="psum", bufs=1, space=bass.MemorySpace.PSUM) as ppool,
        ):
            # Block indicator: blk[p, b] = 1 iff p // 8 == b (sums octets
            # per batch in the matmul below).  Host-provided — engine
            # memsets can only start at partition 0/32/64/96.
            blk = spool.tile([128, BPC], F32)
            nc.sync.dma_start(out=blk[:], in_=blk_d[:])

            # Bulk loads: rows r0..r0+111 of each region through aligned
            # 512 B windows, all on the SWDGE queue.
            mts = []
            for k, (rb, cb) in enumerate(REGIONS):
                r0 = rb * S
                w0, _ = win(cb)
                mt = rpool.tile([128, GR, PADW], F32, tag=f"mt{k}")
                nc.gpsimd.dma_start(
                    out=mt[:], in_=x[:, r0:r0 + OCT * GR, w0:w0 + PADW]
                )
                mts.append(mt)

            # Remainder rows (r0+112), grouped by column block, on the
            # HWDGE queues (parallel to the SWDGE bulk stream).
            rem_tiles = []  # (tile, j, off, k) per region
            for (ra, rb_, rs), cb, ks, qname in REM_GROUPS:
                w0, off = win(cb)
                rt = spool.tile([BPC, len(ks), PADW], F32, tag=f"rem{cb}")
                eng = getattr(nc, qname)
                eng.dma_start(out=rt[:], in_=x[:, ra:rb_:rs, w0:w0 + PADW])
                for j, k in enumerate(ks):
                    rem_tiles.append((rt, j, off, k))

            # Per-region: VectorE reduce of the 113 live columns ->
            # [128,1]; TensorE matmul with blk contracts octets -> PSUM
            # column k.
            py = ppool.tile([BPC, K], F32)
            for k, (rb, cb) in enumerate(REGIONS):
                _, off = win(cb)
                if k == K - 1:
                    # Last region is the latency tail: split its reduce
                    # across VectorE (rows 0..6) and ScalarE (rows 7..13,
                    # Copy-activation with accum) so both halves run
                    # concurrently; the two matmuls accumulate in PSUM.
                    mpv = mpool.tile([128, 1], F32, tag="mpv")
                    nc.vector.reduce_sum(
                        out=mpv[:], in_=mts[k][:, 0:GR // 2, off:off + S],
                        axis=mybir.AxisListType.XY,
                    )
                    scr = mpool.tile([128, GR - GR // 2, S], F32, tag="scr")
                    mpa = mpool.tile([128, 1], F32, tag="mpa")
                    nc.scalar.activation(
                        scr[:], mts[k][:, GR // 2:GR, off:off + S],
                        mybir.ActivationFunctionType.Copy, accum_out=mpa[:],
                    )
                    nc.tensor.matmul(py[:, k:k + 1], blk[:], mpv[:], start=True, stop=False)
                    nc.tensor.matmul(py[:, k:k + 1], blk[:], mpa[:], start=False, stop=True)
                else:
                    mp = mpool.tile([128, 1], F32, tag=f"mp{k}")
                    nc.vector.reduce_sum(
                        out=mp[:], in_=mts[k][:, :, off:off + S],
                        axis=mybir.AxisListType.XY,
                    )
                    nc.tensor.matmul(py[:, k:k + 1], blk[:], mp[:], start=True, stop=True)

            # Remainder-row partials -> rpart[:, k].
            rpart = spool.tile([BPC, K], F32)
            for rt, j, off, k in sorted(rem_tiles, key=lambda t: t[3]):
                nc.vector.reduce_sum(
                    out=rpart[:, k:k + 1], in_=rt[:, j, off:off + S],
                    axis=mybir.AxisListType.X,
                )

            ys = spool.tile([BPC, K], F32)
            nc.vector.tensor_add(ys[:], py[:], rpart[:])

            # Softmax over the 10 detectors, batches on partitions.
            m = spool.tile([BPC, 1], F32)
            nc.vector.reduce_max(m[:], ys[:], axis=mybir.AxisListType.X)
            negm = spool.tile([BPC, 1], F32)
            nc.vector.tensor_scalar_mul(negm[:], m[:], -1.0)
            e = spool.tile([BPC, K], F32)
            ssum = spool.tile([BPC, 1], F32)
            nc.scalar.activation(
                e[:], ys[:], mybir.ActivationFunctionType.Exp,
                bias=negm[:], accum_out=ssum[:],
            )
            rcp = spool.tile([BPC, 1], F32)
            nc.vector.reciprocal(rcp[:], ssum[:])
            o = spool.tile([BPC, K], F32)
            # Per-partition broadcast multiply on DVE (one less engine hop
            # than ScalarE mul before the output DMA).
            nc.vector.tensor_scalar_mul(o[:], e[:], rcp[:])
            nc.sync.dma_start(out=out[:], in_=o[:], single_packet=True)

    nc.compile()
    return nc


_NC = None


def get_nc():
    global _NC
    if _NC is None:
        _NC = build_nc()
    return _NC


def kernel(x, filt=None, **_unused):
    nc = get_nc()
    x = np.ascontiguousarray(np.asarray(x, dtype=np.float32))
    assert x.shape == (B, H, W), x.shape
    blk = np.repeat(np.eye(BPC, dtype=np.float32), OCT, axis=0)
    in_maps = [
        {"x": x[i * BPC:(i + 1) * BPC], "blk": blk} for i in range(N_CORES)
    ]
    res = run_bass_kernel_spmd(nc, in_maps, list(range(N_CORES)))
    return np.concatenate([r["out"] for r in res.results], axis=0)



# revision 2
# speedup vs baseline: 1.1824x; 1.1824x over previous
"""Trainium2 Bass kernel for nn_Detector (region-sum pooling + softmax), v4.

out[b,k] = softmax_k( sum of x[b] over region k ), regions = 10 disjoint
113x113 rectangles of the 1024x1024 image.  Data-parallel over batch:
8 NeuronCores x 16 images.

v4 (from HW DMA microbenchmarks, 8-core load):
- Exact-width 452B descriptors (no 128-col padding): 452B descs run at
  the same ~205-235 GB/s post-ramp as padded 512B ones (no sub-512B RMW
  cliff on HW reads), saving 11% of bytes.  Bulk = 8.10 MB/core.
- single_packet=True on the bulk SWDGE DMAs (free, occasionally helps).
- Remainder row (113th) and blk-indicator DMAs issue BEFORE the bulk so
  their 1-descriptor HWDGE packets ride the queue while Q0 is still
  filling, instead of stealing packet slots mid-stream.
- Last region is loaded as two half-height DMAs so its reduce overlaps
  the stream tail (vector does half 1, scalar-ACT the other half).
- Compute is off the critical path: per-region VectorE reduce ->
  TensorE matmul with a 0/1 octet->batch indicator into PSUM column k;
  ScalarE softmax.  (Measured: compute engines do not slow the stream.)
"""

import numpy as np

import concourse.bass as bass
import concourse.tile as tile
from concourse import bacc, mybir
from concourse.bass_utils import run_bass_kernel_spmd

B, H, W = 128, 1024, 1024
S = 113  # min(1024 // 9, 1024 // 7)
REGIONS = [(2, 1), (2, 4), (2, 7), (4, 1), (4, 3), (4, 5), (4, 7), (6, 1), (6, 4), (6, 7)]
K = len(REGIONS)
N_CORES = 8
BPC = B // N_CORES
F32 = mybir.dt.float32
OCT, GR = 8, 14  # 112 of the 113 region rows = 8 octets x 14 rows

# Remainder-row groups (row r = rb*S + 112), grouped by column block so one
# stepped-slice HWDGE DMA covers regions sharing a column at uniform stride.
#   (row_slice, col_block, [region ks], engine)
REM_GROUPS = [
    ((338, 791, 226), 1, [0, 3, 7], "sync"),
    ((338, 791, 226), 7, [2, 6, 9], "scalar"),
    ((338, 791, 452), 4, [1, 8], "sync"),
    ((564, 565, 1), 3, [4], "scalar"),
    ((564, 565, 1), 5, [5], "sync"),
]


def build_nc():
    nc = bacc.Bacc("TRN2", target_bir_lowering=False, debug=False)
    x = nc.declare_dram_parameter("x", [BPC, H, W], F32, isOutput=False)
    blk_d = nc.declare_dram_parameter("blk", [128, BPC], F32, isOutput=False)
    out = nc.declare_dram_parameter("out", [BPC, K], F32, isOutput=True)

    with tile.TileContext(nc) as tc:
        with (
            tc.tile_pool(name="reg", bufs=1) as rpool,
            tc.tile_pool(name="mp", bufs=1) as mpool,
            tc.tile_pool(name="small", bufs=1) as spool,
            tc.tile_pool(name="psum", bufs=1, space=bass.MemorySpace.PSUM) as ppool,
        ):
            # blk[p, b] = 1 iff p // 8 == b: contracts the 8 row-octets of
            # each batch in the per-region matmul.  Host-provided.
            blk = spool.tile([128, BPC], F32)
            nc.sync.dma_start(out=blk[:], in_=blk_d[:])

            # Remainder rows first (tiny; HWDGE 1-desc packets drain while
            # the bulk queue is still being emitted).
            rem_tiles = []  # (tile, j, k)
            for (ra, rb_, rs), cb, ks, qname in REM_GROUPS:
                c0 = cb * S
                rt = spool.tile([BPC, len(ks), S], F32, name=f"rem{cb}", tag=f"rem{cb}")
                getattr(nc, qname).dma_start(out=rt[:], in_=x[:, ra:rb_:rs, c0:c0 + S])
                for j, k in enumerate(ks):
                    rem_tiles.append((rt, j, k))

            # Bulk: rows r0..r0+111 of each region, exact 452B descriptors.
            # Last region split into two half-height DMAs to shrink the tail.
            mts = []
            for k, (rb, cb) in enumerate(REGIONS):
                r0, c0 = rb * S, cb * S
                mt = rpool.tile([128, GR, S], F32, name=f"mt{k}", tag=f"mt{k}")
                if k == K - 1:
                    nc.gpsimd.dma_start(
                        out=mt[:, 0:GR // 2], in_=x[:, r0:r0 + 56, c0:c0 + S],
                        single_packet=True)
                    nc.gpsimd.dma_start(
                        out=mt[:, GR // 2:GR], in_=x[:, r0 + 56:r0 + 112, c0:c0 + S],
                        single_packet=True)
                else:
                    nc.gpsimd.dma_start(
                        out=mt[:], in_=x[:, r0:r0 + 112, c0:c0 + S],
                        single_packet=True)
                mts.append(mt)

            # Remainder-row partials early (vector, overlapped with stream).
            rpart = spool.tile([BPC, K], F32)
            for rt, j, k in sorted(rem_tiles, key=lambda t: t[2]):
                nc.vector.reduce_sum(
                    out=rpart[:, k:k + 1], in_=rt[:, j, :],
                    axis=mybir.AxisListType.X)

            # Per-region reduce + matmul into PSUM column k.
            py = ppool.tile([BPC, K], F32)
            for k in range(K):
                if k == K - 1:
                    # Tail: vector takes the first half, scalar-ACT the
                    # second; the two matmuls accumulate in PSUM.
                    mpv = mpool.tile([128, 1], F32, tag="mpv")
                    nc.vector.reduce_sum(
                        out=mpv[:], in_=mts[k][:, 0:GR // 2],
                        axis=mybir.AxisListType.XY)
                    scr = mpool.tile([128, GR - GR // 2, S], F32, tag="scr")
                    mpa = mpool.tile([128, 1], F32, tag="mpa")
                    nc.scalar.activation(
                        scr[:], mts[k][:, GR // 2:GR],
                        mybir.ActivationFunctionType.Copy, accum_out=mpa[:])
                    nc.tensor.matmul(py[:, k:k + 1], blk[:], mpv[:], start=True, stop=False)
                    nc.tensor.matmul(py[:, k:k + 1], blk[:], mpa[:], start=False, stop=True)
                else:
                    mp = mpool.tile([128, 1], F32, name=f"mp{k}", tag=f"mp{k}")
                    nc.vector.reduce_sum(
                        out=mp[:], in_=mts[k][:], axis=mybir.AxisListType.XY)
                    nc.tensor.matmul(py[:, k:k + 1], blk[:], mp[:], start=True, stop=True)

            ys = spool.tile([BPC, K], F32)
            nc.vector.tensor_add(ys[:], py[:], rpart[:])

            # Numerically stable softmax over the 10 detectors.
            m = spool.tile([BPC, 1], F32)
            nc.vector.reduce_max(m[:], ys[:], axis=mybir.AxisListType.X)
            negm = spool.tile([BPC, 1], F32)
            nc.vector.tensor_scalar_mul(negm[:], m[:], -1.0)
            e = spool.tile([BPC, K], F32)
            ssum = spool.tile([BPC, 1], F32)
            nc.scalar.activation(
                e[:], ys[:], mybir.ActivationFunctionType.Exp,
                bias=negm[:], accum_out=ssum[:])
            rcp = spool.tile([BPC, 1], F32)
            nc.vector.reciprocal(rcp[:], ssum[:])
            o = spool.tile([BPC, K], F32)
            nc.vector.tensor_scalar_mul(o[:], e[:], rcp[:])
            nc.sync.dma_start(out=out[:], in_=o[:], single_packet=True)

    nc.compile()
    return nc


_NC = None


def get_nc():
    global _NC
    if _NC is None:
        _NC = build_nc()
    return _NC


def kernel(x, filt=None, **_unused):
    nc = get_nc()
    x = np.ascontiguousarray(np.asarray(x, dtype=np.float32))
    assert x.shape == (B, H, W), x.shape
    blk = np.repeat(np.eye(BPC, dtype=np.float32), OCT, axis=0)
    in_maps = [
        {"x": x[i * BPC:(i + 1) * BPC], "blk": blk} for i in range(N_CORES)
    ]
    res = run_bass_kernel_spmd(nc, in_maps, list(range(N_CORES)))
    return np.concatenate([r["out"] for r in res.results], axis=0)


# revision 3
# speedup vs baseline: 1.1969x; 1.0122x over previous
"""Trainium2 Bass kernel for nn_Detector (region-sum pooling + softmax), v5.

out[b,k] = softmax_k( sum of x[b] over region k ), regions = 10 disjoint
113x113 rectangles of the 1024x1024 image.  Data-parallel over batch:
8 NeuronCores x 16 images.

Design (from HW DMA microbenchmarks under full 8-core load):
- The dense [B,HW]x[HW,10] GEMM is a sparse pooling; only 8.10 MB/core
  (the exact 452B-wide region rows) is ever read.  Exact-width 452B
  descriptors run at the same ~200-235 GB/s as padded-512B ones (no
  sub-512B RMW cliff on HBM reads), so no column padding.
- The stream is packet-overhead-bound (~150ns per ~4KB SWDGE packet
  caps it at ~205-235 GB/s; only >4KB descriptors go faster, and any
  wider/contiguous load inflates bytes more than it gains rate).
- SDMA only starts draining a DMA after its full descriptor emission
  (doorbell), so the first bulk DMA is a 2-row sliver to open the
  stream ~1us earlier.
- The last bulk DMA is a 2-row sliver too: the tail after the final
  byte is just a tiny reduce + matmul + softmax.  Region 9 streams as
  6+6+2 rows with incremental PSUM accumulation (vector + scalar-ACT).
- Remainder row (113th) and blk DMAs issue before the bulk so their
  1-descriptor HWDGE packets don't steal packet slots mid-stream.
- Compute (VectorE reduces -> TensorE matmul with a 0/1 octet->batch
  indicator into PSUM; ScalarE softmax) measurably does not slow the
  stream.
"""

import numpy as np

import concourse.bass as bass
import concourse.tile as tile
from concourse import bacc, mybir
from concourse.bass_utils import run_bass_kernel_spmd

B, H, W = 128, 1024, 1024
S = 113  # min(1024 // 9, 1024 // 7)
REGIONS = [(2, 1), (2, 4), (2, 7), (4, 1), (4, 3), (4, 5), (4, 7), (6, 1), (6, 4), (6, 7)]
K = len(REGIONS)
N_CORES = 8
BPC = B // N_CORES
F32 = mybir.dt.float32
OCT, GR = 8, 14  # 112 of the 113 region rows = 8 octets x 14 rows

# Remainder-row groups (row r = rb*S + 112), one stepped-slice HWDGE DMA
# per column block: (row_slice, col_block, [region ks], engine).
REM_GROUPS = [
    ((338, 791, 226), 1, [0, 3, 7], "sync"),
    ((338, 791, 226), 7, [2, 6, 9], "scalar"),
    ((338, 791, 452), 4, [1, 8], "sync"),
    ((564, 565, 1), 3, [4], "scalar"),
    ((564, 565, 1), 5, [5], "sync"),
]


def build_nc():
    nc = bacc.Bacc("TRN2", target_bir_lowering=False, debug=False)
    x = nc.declare_dram_parameter("x", [BPC, H, W], F32, isOutput=False)
    blk_d = nc.declare_dram_parameter("blk", [128, BPC], F32, isOutput=False)
    out = nc.declare_dram_parameter("out", [BPC, K], F32, isOutput=True)

    with tile.TileContext(nc) as tc:
        with (
            tc.tile_pool(name="reg", bufs=1) as rpool,
            tc.tile_pool(name="mp", bufs=1) as mpool,
            tc.tile_pool(name="small", bufs=1) as spool,
            tc.tile_pool(name="psum", bufs=1, space=bass.MemorySpace.PSUM) as ppool,
        ):
            blk = spool.tile([128, BPC], F32)
            nc.sync.dma_start(out=blk[:], in_=blk_d[:])

            # Remainder rows first (tiny HWDGE transfers).
            rem_tiles = []  # (tile, j, k)
            for (ra, rb_, rs), cb, ks, qname in REM_GROUPS:
                c0 = cb * S
                rt = spool.tile([BPC, len(ks), S], F32, name=f"rem{cb}", tag=f"rem{cb}")
                getattr(nc, qname).dma_start(out=rt[:], in_=x[:, ra:rb_:rs, c0:c0 + S])
                for j, k in enumerate(ks):
                    rem_tiles.append((rt, j, k))

            # Bulk SWDGE stream, exact 452B descriptors.  Row-piece plan per
            # region: region 0 = [2,12] (small first doorbell), region 9 =
            # [6,6,2] (small final sliver), others whole.
            pieces = {0: [2, 12], K - 1: [6, 6, 2]}
            mts = []
            for k, (rb, cb) in enumerate(REGIONS):
                r0, c0 = rb * S, cb * S
                mt = rpool.tile([128, GR, S], F32, name=f"mt{k}", tag=f"mt{k}")
                g0 = 0
                for rows in pieces.get(k, [GR]):
                    nc.gpsimd.dma_start(
                        out=mt[:, g0:g0 + rows],
                        in_=x[:, r0 + g0 * OCT:r0 + (g0 + rows) * OCT, c0:c0 + S],
                        single_packet=True)
                    g0 += rows
                mts.append(mt)

            # Remainder-row partials early (overlap the stream).
            rpart = spool.tile([BPC, K], F32)
            for rt, j, k in sorted(rem_tiles, key=lambda t: t[2]):
                nc.vector.reduce_sum(
                    out=rpart[:, k:k + 1], in_=rt[:, j, :],
                    axis=mybir.AxisListType.X)

            # Per-region reduce -> matmul into PSUM column k.
            py = ppool.tile([BPC, K], F32)
            for k in range(K):
                if k == K - 1:
                    # 6 rows on vector, 6 on scalar-ACT, final 2-row sliver
                    # on vector; three matmuls accumulate in PSUM.
                    mpv = mpool.tile([128, 1], F32, tag="mpv")
                    nc.vector.reduce_sum(
                        out=mpv[:], in_=mts[k][:, 0:6], axis=mybir.AxisListType.XY)
                    scr = mpool.tile([128, 6, S], F32, tag="scr")
                    mpa = mpool.tile([128, 1], F32, tag="mpa")
                    nc.scalar.activation(
                        scr[:], mts[k][:, 6:12],
                        mybir.ActivationFunctionType.Copy, accum_out=mpa[:])
                    mpc = mpool.tile([128, 1], F32, tag="mpc")
                    nc.vector.reduce_sum(
                        out=mpc[:], in_=mts[k][:, 12:GR], axis=mybir.AxisListType.XY)
                    nc.tensor.matmul(py[:, k:k + 1], blk[:], mpv[:], start=True, stop=False)
                    nc.tensor.matmul(py[:, k:k + 1], blk[:], mpa[:], start=False, stop=False)
                    nc.tensor.matmul(py[:, k:k + 1], blk[:], mpc[:], start=False, stop=True)
                else:
                    mp = mpool.tile([128, 1], F32, name=f"mp{k}", tag=f"mp{k}")
                    nc.vector.reduce_sum(
                        out=mp[:], in_=mts[k][:], axis=mybir.AxisListType.XY)
                    nc.tensor.matmul(py[:, k:k + 1], blk[:], mp[:], start=True, stop=True)

            ys = spool.tile([BPC, K], F32)
            nc.vector.tensor_add(ys[:], py[:], rpart[:])

            # Stable softmax; negated max fused into the reduce.
            negm = spool.tile([BPC, 1], F32)
            nc.vector.tensor_reduce(
                out=negm[:], in_=ys[:], axis=mybir.AxisListType.X,
                op=mybir.AluOpType.max, negate=True)
            e = spool.tile([BPC, K], F32)
            ssum = spool.tile([BPC, 1], F32)
            nc.scalar.activation(
                e[:], ys[:], mybir.ActivationFunctionType.Exp,
                bias=negm[:], accum_out=ssum[:])
            rcp = spool.tile([BPC, 1], F32)
            nc.vector.reciprocal(rcp[:], ssum[:])
            o = spool.tile([BPC, K], F32)
            nc.vector.tensor_scalar_mul(o[:], e[:], rcp[:])
            nc.sync.dma_start(out=out[:], in_=o[:], single_packet=True)

    nc.compile()
    return nc


_NC = None


def get_nc():
    global _NC
    if _NC is None:
        _NC = build_nc()
    return _NC


def kernel(x, filt=None, **_unused):
    nc = get_nc()
    x = np.ascontiguousarray(np.asarray(x, dtype=np.float32))
    assert x.shape == (B, H, W), x.shape
    blk = np.repeat(np.eye(BPC, dtype=np.float32), OCT, axis=0)
    in_maps = [
        {"x": x[i * BPC:(i + 1) * BPC], "blk": blk} for i in range(N_CORES)
    ]
    res = run_bass_kernel_spmd(nc, in_maps, list(range(N_CORES)))
    return np.concatenate([r["out"] for r in res.results], axis=0)
